# revision 1
# baseline (speedup 1.0000x reference)
"""CenterLoss Trainium2 kernel.

Reference computes, for x[B,D], labels[B], centers[C,D]:
    distmat[b,c] = ||x_b||^2 + ||c_c||^2 - 2<x_b, c_c>
    dist = where(labels[b]==c, distmat, 0)
    loss = clip(dist, 1e-12, 1e12).sum() / B

Only one entry per row survives the mask: d_b = ||x_b - centers[labels_b]||^2.
The other C-1 zeros per row are clamped to 1e-12, contributing the constant
B*(C-1)*1e-12 to the sum.  So:

    loss = ( sum_b clip(d_b, 1e-12, 1e12) ) / B  +  (C-1)*1e-12

No [B,C] distmat needed: gather centers[labels] (indirect DMA), squared
distance per row (scaled by 1/B, with the clip bounds scaled to match),
clip, reduce.  Data-parallel over batch across 8 cores; centers stay in
HBM and only the labeled rows are read (indirect gather).

Raw bacc, no Tile, no Block: engine programs are emitted straight into the
main basic block (single IRAM block, no body ifetch, no exit barrier) with
manual semaphores.  Only Sync (input/output DMA), GpSimd (gather + final
cross-partition reduce) and Vector are used; the Tensor engine is unused so
its preamble (a config write plus a ~2.4us settle that gates the entry
barrier) is skipped.

Per-core layout: row r of the 1024-row shard lives at partition p = r//8,
free slot j = r%8 (x and label loads are contiguous DMAs; gather j fetches
rows {p*8+j} via per-partition offsets it[:, j]).
"""

import numpy as np

B, C, D = 8192, 10000, 128
N_CORES = 8
RPC = B // N_CORES  # rows per core
P = 128
J = RPC // P  # free slots per partition

CLIP_LO = 1e-12
CLIP_HI = 1e12
MASK_CONST = (C - 1) * CLIP_LO  # clamped masked-out zeros, after /B

_cache = {}


def _build():
    from contextlib import ExitStack

    import concourse.bacc as bacc
    import concourse.bass as bass
    import concourse.mybir as mybir

    f32 = mybir.dt.float32
    i32 = mybir.dt.int32

    class _FastBacc(bacc.Bacc):
        # the init-time all-engine barrier only guards the const-ap
        # memsets, which this kernel never reads — skip it
        def all_engine_barrier(self, **kw):
            return

    # PE is unused; its preamble's config-write + settle would gate the
    # runtime entry barrier for ~2.8us
    pe_preamble = bass.BassTensorEngine.preamble
    bass.BassTensorEngine.preamble = lambda self: None
    try:
        nc = _FastBacc("TRN2", target_bir_lowering=False, debug=False)
    finally:
        bass.BassTensorEngine.preamble = pe_preamble

    x_d = nc.dram_tensor("x", [RPC, D], f32, kind="ExternalInput")
    lab_d = nc.dram_tensor("labels", [P, J], i32, kind="ExternalInput")
    cen_d = nc.dram_tensor("centers", [C, D], f32, kind="ExternalInput")
    out_d = nc.dram_tensor("out", [1, 1], f32, kind="ExternalOutput")

    with (
        ExitStack() as ctx,
        nc.sbuf_tensor("xt", [P, J, D], f32) as xt,
        nc.sbuf_tensor("ct", [P, J, D], f32) as ct,
        nc.sbuf_tensor("sq", [P, J, D], f32) as sq,
        nc.sbuf_tensor("sq2", [P, J, D], f32) as sq2,
        nc.sbuf_tensor("it", [P, J], i32) as it,
        nc.sbuf_tensor("dsum", [P, J], f32) as dsum,
        nc.sbuf_tensor("dclip", [P, J], f32) as dclip,
        nc.sbuf_tensor("dtot", [P, 1], f32) as dtot,
        nc.sbuf_tensor("res", [1, 1], f32) as res,
        nc.semaphore("s_idx") as s_idx,
        nc.semaphore("s_x") as s_x,
        nc.semaphore("s_v") as s_v,
        nc.semaphore("s_r") as s_r,
        nc.semaphore("s_out") as s_out,
    ):
        s_g = [ctx.enter_context(nc.semaphore(f"s_g{j}")) for j in range(J)]  # noqa: ANT232

        # ---- Sync: idx DMA strictly first (its receipt gates the gathers),
        # then x with contiguous 4KB-per-partition descriptors
        nc.sync.dma_start(out=it[:], in_=lab_d[:, :]).then_inc(s_idx, 16)
        x_ap = x_d[:, :].rearrange("(p j) d -> p (j d)", p=P)
        nc.sync.dma_start(
            out=xt[:].rearrange("p j d -> p (j d)"), in_=x_ap
        ).then_inc(s_x, 16)
        nc.sync.wait_ge(s_r, 1)
        nc.sync.dma_start(out=out_d[:, :], in_=res[:]).then_inc(s_out, 16)
        nc.sync.wait_ge(s_out, 16)

        # ---- GpSimd: per-slot indirect gathers, then the cross-partition sum
        nc.gpsimd.wait_ge(s_idx, 16)
        for j in range(J):
            nc.gpsimd.indirect_dma_start(
                out=ct[:, j, :],
                out_offset=None,
                in_=cen_d[:, :],
                in_offset=bass.IndirectOffsetOnAxis(ap=it[:, j : j + 1], axis=0),
            ).then_inc(s_g[j], 16)
        nc.gpsimd.wait_ge(s_v, 1)
        nc.gpsimd.tensor_reduce(
            out=res[:],
            in_=dtot[:],
            axis=mybir.AxisListType.C,
            op=mybir.AluOpType.add,
        ).then_inc(s_r, 1)

        # ---- Vector: per-tile (x-c), then (x-c)^2/B with fused row-sum
        nc.vector.wait_ge(s_x, 16)
        for j in range(J):
            nc.vector.wait_ge(s_g[j], 16)
            nc.vector.tensor_tensor(
                out=sq[:, j, :],
                in0=xt[:, j, :],
                in1=ct[:, j, :],
                op=mybir.AluOpType.subtract,
            )
            nc.vector.drain()  # DVE pipeline: sq_j write -> read below
            nc.vector.scalar_tensor_tensor(
                out=sq2[:, j, :],
                in0=sq[:, j, :],
                scalar=1.0 / B,
                in1=sq[:, j, :],
                op0=mybir.AluOpType.mult,
                op1=mybir.AluOpType.mult,
                accum_out=dsum[:, j : j + 1],
            )
        nc.vector.drain()
        nc.vector.tensor_scalar(
            out=dclip[:],
            in0=dsum[:],
            scalar1=CLIP_LO / B,
            scalar2=CLIP_HI / B,
            op0=mybir.AluOpType.max,
            op1=mybir.AluOpType.min,
        )
        nc.vector.drain()
        nc.vector.tensor_reduce(
            out=dtot[:],
            in_=dclip[:],
            axis=mybir.AxisListType.X,
            op=mybir.AluOpType.add,
        )
        nc.vector.drain().then_inc(s_v, 1)

    nc.compile()
    return nc


def _get_nc():
    if "nc" not in _cache:
        _cache["nc"] = _build()
    return _cache["nc"]


def _make_in_maps(x, labels, centers):
    x = np.ascontiguousarray(np.asarray(x, dtype=np.float32))
    labels = np.asarray(labels).astype(np.int32)
    centers = np.ascontiguousarray(np.asarray(centers, dtype=np.float32))
    in_maps = []
    for i in range(N_CORES):
        sl = slice(i * RPC, (i + 1) * RPC)
        in_maps.append(
            {
                "x": x[sl],
                "labels": np.ascontiguousarray(labels[sl].reshape(P, J)),
                "centers": centers,
            }
        )
    return in_maps


def _run(in_maps, trace=False, **kwargs):
    from concourse.bass_utils import run_bass_kernel_spmd

    nc = _get_nc()
    return run_bass_kernel_spmd(
        nc, in_maps, core_ids=list(range(N_CORES)), trace=trace, **kwargs
    )


def kernel(x, labels, centers):
    res = _run(_make_in_maps(x, labels, centers))
    total = np.float32(0.0)
    for r in res.results:
        total += np.float32(r["out"].reshape(()))
    return np.asarray(total + np.float32(MASK_CONST), dtype=np.float32)

